# revision 23
# baseline (speedup 1.0000x reference)
"""Trainium2 Bass kernel: multi-head attention (B=4, N=1024, D=1024, H=16)
distributed over 8 NeuronCores.

Sharding: core = 2*b + g takes batch b (of 4) and head-group g (of 2, i.e.
8 heads = 512 qkv features). Each core computes Q/K/V projections for its
feature slice over all 1024 tokens, full attention for its 8 heads, and a
PARTIAL output projection (contraction over its 512 features). The two
partial [1024, 1024] products per batch are summed (+bias) on the host.

Schedule: score matmuls for each head pair are row-packed (K=64 tiles at
array rows 0-63 / 64-127, concurrent); exp is batched over two PSUM banks
per ACT instruction; projection + early output-projection waves thread
through the attention stream as fillers so the PE never idles while the
ACT engine works through the exps.
"""

import numpy as np
import concourse.bacc as bacc
import concourse.mybir as mybir
import concourse.tile as tile

dt = mybir.dt
F32, BF16 = dt.float32, dt.bfloat16

B, N, D = 4, 1024, 1024
H_TOT, DH = 16, 64
HG = 8              # heads per core
FG = 512            # qkv features per core (head-group)
P = 128
DC = D // P         # 8 contraction chunks over x features
ET = FG // P        # 4 feature chunks per group
NT = N // P         # 8 key-token tiles
NQH = 512           # queries per psum half
SCALE = DH ** -0.5
AF = mybir.ActivationFunctionType


def _build_nc():
    nc = bacc.Bacc("TRN2", target_bir_lowering=False, debug=False)
    xT = nc.dram_tensor("xT", [D, N], BF16, kind="ExternalInput")
    wqT = nc.dram_tensor("wqT", [D, FG], BF16, kind="ExternalInput")
    wkT = nc.dram_tensor("wkT", [D, FG], BF16, kind="ExternalInput")
    wvT = nc.dram_tensor("wvT", [D, FG], BF16, kind="ExternalInput")
    woutT = nc.dram_tensor("woutT", [FG, D], BF16, kind="ExternalInput")
    y = nc.dram_tensor("y", [N, D], F32, kind="ExternalOutput")

    with tile.TileContext(nc) as tc:
        with (
            tc.tile_pool(name="const", bufs=1) as cp,
            tc.tile_pool(name="work", bufs=2) as wp,
            tc.tile_pool(name="ps", bufs=1, space="PSUM") as pp,
        ):
            x_sb = cp.tile([P, DC, N], BF16)
            wq_sb = cp.tile([P, DC, FG], BF16)
            wk_sb = cp.tile([P, DC, FG], BF16)
            wv_sb = cp.tile([P, DC, FG], BF16)
            wout_sb = cp.tile([P, ET, D], BF16)
            # x chunk-wise on the sync queue; weights whole on the gpsimd
            # queue so descriptor generation runs in parallel.
            # first-needed slices first: wq/wk et0 columns, then x chunks
            nc.sync.dma_start(wq_sb[:, :, 0:P],
                              wqT.ap()[:, 0:P].rearrange("(c p) n -> p c n", p=P))
            nc.sync.dma_start(wk_sb[:, :, 0:P],
                              wkT.ap()[:, 0:P].rearrange("(c p) n -> p c n", p=P))
            for c in range(DC):
                nc.sync.dma_start(x_sb[:, c, :], xT.ap()[c * P:(c + 1) * P, :])
            nc.sync.dma_start(wv_sb[:, :, :],
                              wvT.ap().rearrange("(c p) n -> p c n", p=P))
            nc.sync.dma_start(wq_sb[:, :, P:FG],
                              wqT.ap()[:, P:FG].rearrange("(c p) n -> p c n", p=P))
            nc.sync.dma_start(wk_sb[:, :, P:FG],
                              wkT.ap()[:, P:FG].rearrange("(c p) n -> p c n", p=P))
            nc.sync.dma_start(wout_sb[:, :, :],
                              woutT.ap().rearrange("(c p) n -> p c n", p=P))

            ones64 = cp.tile([1, DH], BF16)
            nc.vector.memset(ones64, 1.0)
            junkA = cp.tile([P, P], BF16)
            junkB = cp.tile([P, 256], BF16)
            nc.vector.memset(junkA, 0.0)
            nc.vector.memset(junkB, 0.0)

            q_sb = cp.tile([P, ET, N], BF16)
            k_sb = cp.tile([P, ET, N], BF16)
            v_sb = cp.tile([P, NT, HG, DH + 1], BF16)
            nc.vector.memset(v_sb[:, :, :, DH:DH + 1], 1.0)
            aT_sb = cp.tile([P, ET, N], BF16)

            # HAM warmup: dummy matmuls with no input deps keep the PE busy
            # (and un-throttled) while the first DMAs land.
            warm_ps = pp.tile([P, 256], F32, tag="proj", bufs=2, name="warm")
            for w in range(20):
                nc.tensor.matmul(warm_ps[:, :], lhsT=junkA[:, :], rhs=junkB[:, :],
                                 start=True, stop=True)

            # ---- projection waves as resumable single-matmul steps
            def qk_steps(w_sb, out_sb, et, j):
                state = {}
                def step(c):
                    if c == 0:
                        state["ps"] = pp.tile([P, NQH], F32, tag="proj", bufs=2,
                                              name=f"pp{id(w_sb)}_{et}_{j}")
                    nc.tensor.matmul(
                        state["ps"][:, :],
                        lhsT=w_sb[:, c, et * P:(et + 1) * P],
                        rhs=x_sb[:, c, j * NQH:(j + 1) * NQH],
                        start=(c == 0), stop=(c == DC - 1),
                    )
                    if c == DC - 1:
                        nc.vector.tensor_copy(
                            out_sb[:, et, j * NQH:(j + 1) * NQH], state["ps"][:, :])
                return [lambda c=c: step(c) for c in range(DC)]

            def v_steps(nt):
                state = {}
                def step(c):
                    if c == 0:
                        state["ps"] = pp.tile([P, NQH], F32, tag="proj", bufs=2,
                                              name=f"vps{nt}")
                    nc.tensor.matmul(
                        state["ps"][:, :],
                        lhsT=x_sb[:, c, nt * P:(nt + 1) * P],
                        rhs=wv_sb[:, c, :],
                        start=(c == 0), stop=(c == DC - 1),
                    )
                    if c == DC - 1:
                        nc.vector.tensor_copy(
                            v_sb[:, nt, :, 0:DH],
                            state["ps"][:, :].rearrange("p (h d) -> p h d", h=HG),
                        )
                return [lambda c=c: step(c) for c in range(DC)]

            # ---- deferred per-head normalization
            pending_norm = []

            def finish_head(t, j, r, pv, sr):
                # NOTE: bc rides the "s" pool, not "proj" — proj buffers hold
                # long-lived accumulating waves and an interleaved allocation
                # would rotate onto (and clobber) a live accumulator.
                bc_ps = pp.tile([DH, NQH], F32, tag="s", bufs=2,
                                name=f"bc{t}_{j}_{r}")
                nc.tensor.matmul(bc_ps[:, :], lhsT=ones64[:, :], rhs=sr[:, :],
                                 start=True, stop=True)
                bc_sb = wp.tile([DH, NQH], F32, tag="bc_sb", bufs=2,
                                name=f"bcs{t}_{j}_{r}")
                nc.vector.tensor_copy(bc_sb[:, :], bc_ps[:, :])
                nc.vector.tensor_mul(
                    aT_sb[r:r + DH, t, j * NQH:(j + 1) * NQH],
                    pv[0:DH, :], bc_sb[:, :])

            # ---- filler queue
            filler_steps = []

            def add_unit(deadline, steps):
                for s in steps:
                    filler_steps.append((deadline, s))
            fill_pos = 0

            def flush_fillers(d):
                nonlocal fill_pos
                while fill_pos < len(filler_steps) and filler_steps[fill_pos][0] <= d:
                    filler_steps[fill_pos][1]()
                    fill_pos += 1

            def pop_filler(n):
                nonlocal fill_pos
                k = 0
                while k < n and fill_pos < len(filler_steps):
                    filler_steps[fill_pos][1]()
                    fill_pos += 1
                    k += 1

            # ---- attention unit: head pair t, query half j. PV matmuls for
            # early key chunks interleave with the later score chunks so the
            # unit fills its own exp-latency bubbles.
            def unit(t, j, u):
                pTA = wp.tile([P, NT, NQH], BF16, tag="pTA", bufs=2, name=f"pTA{u}")
                pTB = wp.tile([P, NT, NQH], BF16, tag="pTB", bufs=2, name=f"pTB{u}")
                flush_fillers(u)

                def scores_kk(kk):
                    sA = pp.tile([P, 2, NQH], F32, tag="s", bufs=2, name=f"sA{u}_{kk}")
                    sB = pp.tile([P, 2, NQH], F32, tag="s", bufs=2, name=f"sB{u}_{kk}")
                    for i in range(2):
                        kc = 2 * kk + i
                        nc.tensor.matmul(
                            sA[:, i, :],
                            lhsT=k_sb[0:DH, t, kc * P:(kc + 1) * P],
                            rhs=q_sb[0:DH, t, j * NQH:(j + 1) * NQH],
                            start=True, stop=True, tile_position=(0, 0),
                        )
                        nc.tensor.matmul(
                            sB[:, i, :],
                            lhsT=k_sb[DH:P, t, kc * P:(kc + 1) * P],
                            rhs=q_sb[DH:P, t, j * NQH:(j + 1) * NQH],
                            start=True, stop=True, tile_position=(DH, 0),
                        )
                        pop_filler(1)
                    nc.scalar.activation(
                        pTA[:, 2 * kk:2 * kk + 2, :], sA[:, :, :], AF.Exp, scale=SCALE)
                    nc.scalar.activation(
                        pTB[:, 2 * kk:2 * kk + 2, :], sB[:, :, :], AF.Exp, scale=SCALE)
                    pop_filler(1)

                scores_kk(0)
                if pending_norm:
                    pending_norm.pop(0)()
                scores_kk(1)
                if pending_norm:
                    pending_norm.pop(0)()
                scores_kk(2)
                # force V-projection waves (unit 0) so every v_sb writer is
                # emitted before the PV matmuls that read it
                flush_fillers(u + 0.4)
                # pv banks recycle the previous unit's — safe now that both
                # previous norms are emitted
                pvA = pp.tile([DH + 1, NQH], F32, tag="pv", bufs=2, name=f"pvA{u}")
                pvB = pp.tile([DH + 1, NQH], F32, tag="pv", bufs=2, name=f"pvB{u}")
                for kc in range(4):
                    nc.tensor.matmul(
                        pvA[:, :], lhsT=v_sb[:, kc, 2 * t, :], rhs=pTA[:, kc, :],
                        start=(kc == 0), stop=False,
                    )
                scores_kk(3)
                flush_fillers(u + 0.5)
                for kc in range(4, NT):
                    nc.tensor.matmul(
                        pvA[:, :], lhsT=v_sb[:, kc, 2 * t, :], rhs=pTA[:, kc, :],
                        start=False, stop=(kc == NT - 1),
                    )
                    pop_filler(1)
                for kc in range(NT):
                    nc.tensor.matmul(
                        pvB[:, :], lhsT=v_sb[:, kc, 2 * t + 1, :], rhs=pTB[:, kc, :],
                        start=(kc == 0), stop=(kc == NT - 1),
                    )
                    pop_filler(1)
                # reciprocal_approx_fast mis-reads PSUM at base_partition 64;
                # stage the denominator row through a partition-0 SBUF tile.
                dnA = wp.tile([1, NQH], F32, tag="dnA", bufs=2, name=f"dnA{u}")
                dnB = wp.tile([1, NQH], F32, tag="dnB", bufs=2, name=f"dnB{u}")
                nc.vector.tensor_copy(dnA[:, :], pvA[DH:DH + 1, :])
                nc.vector.tensor_copy(dnB[:, :], pvB[DH:DH + 1, :])
                srA = wp.tile([1, NQH], F32, tag="srA", bufs=2, name=f"srA{u}")
                srB = wp.tile([1, NQH], F32, tag="srB", bufs=2, name=f"srB{u}")
                nc.vector.reciprocal_approx_fast(srA[:, :], dnA[:, :])
                nc.vector.reciprocal_approx_fast(srB[:, :], dnB[:, :])
                srAb = wp.tile([1, NQH], BF16, tag="srAb", bufs=2, name=f"srAb{u}")
                srBb = wp.tile([1, NQH], BF16, tag="srBb", bufs=2, name=f"srBb{u}")
                nc.vector.tensor_copy(srAb[:, :], srA[:, :])
                nc.vector.tensor_copy(srBb[:, :], srB[:, :])
                pending_norm.append(
                    lambda t=t, j=j, pvA=pvA, srAb=srAb:
                        finish_head(t, j, 0, pvA, srAb))
                pending_norm.append(
                    lambda t=t, j=j, pvB=pvB, srBb=srBb:
                        finish_head(t, j, DH, pvB, srBb))

            # ---- output projection steps (per t4: both jj halves + dma).
            # Split into accumulation (c0..c2) and finalization (c3 + evict
            # + dma) so a tail group's early chunks can run as fillers while
            # the last attention unit is still in flight. The c3 step needs
            # this group's last aT writers already EMITTED: groups t4 0-3
            # enqueue after unit 4 (whose kk-top pops emit unit 3's norms);
            # tail finalizations run after the post-unit norm drain.
            out_state = {}

            def out_accum_steps(t4, c_hi=ET - 1):
                st = out_state.setdefault(t4, {})
                def step(jj, c):
                    if c == 0:
                        if jj == 0:
                            st["y"] = wp.tile([P, D], F32, tag="y_sb", bufs=2,
                                              name=f"ysb{t4}")
                        st[f"ps{jj}"] = pp.tile([P, NQH], F32, tag="proj", bufs=2,
                                                name=f"yps{t4}_{jj}")
                    nc.tensor.matmul(
                        st[f"ps{jj}"][:, :],
                        lhsT=aT_sb[:, c, t4 * P:(t4 + 1) * P],
                        rhs=wout_sb[:, c, jj * NQH:(jj + 1) * NQH],
                        start=(c == 0), stop=False,
                    )
                return [lambda jj=jj, c=c: step(jj, c)
                        for jj in range(2) for c in range(c_hi + 1)]

            def out_final_steps(t4, evict_on_scalar=False):
                st = out_state[t4]
                def step(jj):
                    nc.tensor.matmul(
                        st[f"ps{jj}"][:, :],
                        lhsT=aT_sb[:, ET - 1, t4 * P:(t4 + 1) * P],
                        rhs=wout_sb[:, ET - 1, jj * NQH:(jj + 1) * NQH],
                        start=False, stop=True,
                    )
                    if evict_on_scalar:
                        # ACT is idle after the last exp; keep the DVE free
                        # for the norm muls the tail groups depend on
                        nc.scalar.copy(
                            st["y"][:, jj * NQH:(jj + 1) * NQH], st[f"ps{jj}"][:, :])
                    else:
                        nc.vector.tensor_copy(
                            st["y"][:, jj * NQH:(jj + 1) * NQH], st[f"ps{jj}"][:, :])
                    if jj == 1:
                        nc.sync.dma_start(y.ap()[t4 * P:(t4 + 1) * P, :],
                                          st["y"][:, :])
                return [lambda jj=jj: step(jj) for jj in range(2)]

            def out_steps(t4, evict_on_scalar=False):
                steps = out_accum_steps(t4, ET - 2)
                fin = out_final_steps(t4, evict_on_scalar)
                # interleave: jj0 c0-2, jj0 c3+evict, jj1 c0-2, jj1 c3+evict
                return (steps[0:ET - 1] + [fin[0]] +
                        steps[ET - 1:] + [fin[1]])

            # ---- emit
            for s in qk_steps(wq_sb, q_sb, 0, 0):
                s()
            for jj in range(2):
                for s in qk_steps(wk_sb, k_sb, 0, jj):
                    s()
            for nt in range(NT):
                add_unit(0.3 + nt * 0.01, v_steps(nt))
            for t in range(1, ET):
                add_unit(t - 0.4, qk_steps(wq_sb, q_sb, t, 0))
                add_unit(t - 0.4, qk_steps(wk_sb, k_sb, t, 0))
                add_unit(t - 0.3, qk_steps(wk_sb, k_sb, t, 1))
            for t in range(ET):
                add_unit(3.6 + t, qk_steps(wq_sb, q_sb, t, 1))
            for u, (j, t) in enumerate((j, t) for j in range(2) for t in range(ET)):
                unit(t, j, u)
                if u == 4:
                    # early output-projection groups: t4 0-3 read only j=0
                    # columns of aT, fully written once unit 3's norms are
                    # emitted (done inside unit 4). They become late fillers.
                    for t4 in range(4):
                        add_unit(98, out_steps(t4))
            while pending_norm:
                pending_norm.pop(0)()
            flush_fillers(99)
            for t4 in range(4, N // P):
                for s in out_steps(t4, evict_on_scalar=True):
                    s()
    nc.compile()
    return nc


def _make_in_maps(x, w_qkv, w_out, b_out):
    import ml_dtypes
    bf = ml_dtypes.bfloat16
    wq = [np.ascontiguousarray(w_qkv[g * FG:(g + 1) * FG, :].astype(bf).T)
          for g in range(2)]
    wk = [np.ascontiguousarray(w_qkv[D + g * FG:D + (g + 1) * FG, :].astype(bf).T)
          for g in range(2)]
    wv = [np.ascontiguousarray(w_qkv[2 * D + g * FG:2 * D + (g + 1) * FG, :].astype(bf).T)
          for g in range(2)]
    wo = [np.ascontiguousarray(w_out[:, g * FG:(g + 1) * FG].astype(bf).T)
          for g in range(2)]
    in_maps = []
    for core in range(8):
        b, g = core // 2, core % 2
        in_maps.append({
            "xT": np.ascontiguousarray(x[b].astype(bf).T),
            "wqT": wq[g],
            "wkT": wk[g],
            "wvT": wv[g],
            "woutT": wo[g],
        })
    return in_maps


def _assemble(results, b_out):
    y = np.empty((B, N, D), dtype=np.float32)
    for b in range(B):
        y[b] = results[2 * b]["y"] + results[2 * b + 1]["y"]
    y += b_out.astype(np.float32).reshape(1, 1, D)
    return y


_NC_CACHE = {}


def kernel(x, w_qkv, w_out, b_out):
    import numpy as _np
    from concourse.bass_utils import run_bass_kernel_spmd
    if "nc" not in _NC_CACHE:
        _NC_CACHE["nc"] = _build_nc()
    nc = _NC_CACHE["nc"]
    x, w_qkv = _np.asarray(x), _np.asarray(w_qkv)
    w_out, b_out = _np.asarray(w_out), _np.asarray(b_out)
    in_maps = _make_in_maps(x, w_qkv, w_out, b_out)
    res = run_bass_kernel_spmd(nc, in_maps, list(range(8)))
    return _assemble(res.results, b_out)
